# revision 1
# baseline (speedup 1.0000x reference)
"""Trainium2 Bass kernel: causal multi-head attention with interleaved RoPE.

Problem shapes (hardcoded): x [2, 2048, 1024], 16 heads of dk=64.
Sharding: 8 cores = 2 batches x 4 head-groups (4 heads each). Each core
computes its head-slice Q/K/V projections, RoPE, causal attention, and a
partial output through its Wo row-slice; the host sums the 4 partials per
batch and adds bo.

RoPE trick: attention scores are invariant to any permutation of the dk
axis applied to both Q and K, so the Wq/Wk columns are permuted on the host
into a "quadrant half-split" layout where each rotation pair partner sits
exactly 16 partitions away inside the same 32-partition quadrant. The DVE
stream_shuffle (a per-quadrant 32-way permute) then produces the swapped
operand, and RoPE becomes: rot = q * cosT + shuffle(q) * sinT with
host-precomputed tables (sinT carries the sign).
"""

import os
from contextlib import ExitStack

import numpy as np

import concourse.bass as bass
import concourse.mybir as mybir
import concourse.tile as tile

B, S, D, H = 2, 2048, 1024, 16
DK = D // H  # 64
HG = 4  # heads per core
NCOLS = HG * DK  # 256 columns of the projection per core
THETA = 10000.0
SCALE = 1.0 / float(np.sqrt(DK))
N_CORES = 8

F32 = mybir.dt.float32
F32R = mybir.dt.float32r

# matmul operand dtype: float32r (= TF32, 10-bit mantissa) streams 1 col/cycle
# on the PE vs 4 for float32. Operands must be *rounded* to TF32: DMA-fed
# tensors are pre-rounded on the host and declared float32r; on-chip operand
# producers write float32r directly. Numerics validated in test.py.
USE_F32R = os.environ.get("KERNEL_F32", "0") != "1"
MMDT = F32R if USE_F32R else F32


def round_tf32(a):
    """Round fp32 array to TF32 (RNE to 10-bit mantissa)."""
    if not USE_F32R:
        return np.ascontiguousarray(a, dtype=np.float32)
    u = np.ascontiguousarray(a, dtype=np.float32).view(np.uint32).copy()
    u += 0x0FFF + ((u >> 13) & 1)
    u &= np.uint32(0xFFFFE000)
    return u.view(np.float32)


# ---------------------------------------------------------------------------
# host-side prep
# ---------------------------------------------------------------------------

def _rope_perm():
    """Within-head column permutation pi: new row r -> original dk index."""
    perm = np.empty(DK, dtype=np.int64)
    for r in range(DK):
        q, m = divmod(r, 32)
        if m < 16:
            perm[r] = 2 * (16 * q + m)
        else:
            perm[r] = 2 * (16 * q + m - 16) + 1
    return perm


_PERM = _rope_perm()
SHUF_MASK = list(range(16, 32)) + list(range(16))  # swap 16-halves per quadrant


def _rope_tables(pos):
    """cosT/sinT [128, S] fp32 for the permuted layout. pos: [S] int."""
    inv_freq = (np.float32(THETA) ** (-(np.arange(0, DK, 2, dtype=np.float32) / np.float32(DK))))  # [32]
    ang = pos.astype(np.float32)[:, None] * inv_freq[None, :]  # [S, 32]
    cos = np.cos(ang)  # [S, 32]
    sin = np.sin(ang)
    cosT = np.empty((128, S), dtype=np.float32)
    sinT = np.empty((128, S), dtype=np.float32)
    for p in range(128):
        r = p % DK
        q, m = divmod(r, 32)
        if m < 16:
            i = 16 * q + m
            sgn = -1.0
        else:
            i = 16 * q + m - 16
            sgn = 1.0
        cosT[p] = cos[:, i]
        sinT[p] = np.float32(sgn) * sin[:, i]
    return cosT, sinT


def make_core_inputs(x, token_position, Wq, bq, Wk, bk, Wv, bv, Wo, bo):
    """Build the 8 per-core input maps."""
    x = np.asarray(x, dtype=np.float32)
    token_position = np.asarray(token_position)
    Wq, Wk, Wv, Wo = (np.asarray(w, dtype=np.float32) for w in (Wq, Wk, Wv, Wo))
    bq, bk, bv = (np.asarray(b_, dtype=np.float32) for b_ in (bq, bk, bv))

    in_maps = []
    tables = {}
    for c in range(N_CORES):
        b, hg = divmod(c, HG)
        heads = range(HG * hg, HG * hg + HG)
        # permuted q/k column indices for this core's heads
        cols_qk = np.concatenate([DK * h + _PERM for h in heads])
        cols_v = np.arange(NCOLS * hg, NCOLS * hg + NCOLS)
        if b not in tables:
            tables[b] = _rope_tables(np.asarray(token_position[b]))
        cosT, sinT = tables[b]
        wo_rows = Wo[cols_v, :]  # [256, 1024]
        in_maps.append({
            "xT": round_tf32(x[b].T),                               # [1024, 2048]
            "wq": round_tf32(Wq[:, cols_qk]),                       # [1024, 256]
            "wk": round_tf32(Wk[:, cols_qk]),
            "wv": round_tf32(Wv[:, cols_v]),
            "wo": round_tf32(wo_rows.reshape(HG, DK, D).transpose(1, 0, 2)),  # [64, 4, 1024]
            "bq": round_tf32(bq[cols_qk][None, :]),                 # [1, 256]
            "bk": round_tf32(bk[cols_qk][None, :]),
            "bv": round_tf32(bv[cols_v][None, :]),
            "ones_row": round_tf32(np.ones((1, 512), np.float32)),
            "onesc": round_tf32(np.ones((128, 64), np.float32)),
            "cosT": cosT,
            "sinT": sinT,
        })
    return in_maps


# ---------------------------------------------------------------------------
# device program
# ---------------------------------------------------------------------------

def build_program(with_bias=False):
    from concourse import bacc, library_config
    nc = bacc.Bacc("TRN2", debug=False)

    xT = nc.declare_dram_parameter("xT", [D, S], MMDT, isOutput=False).ap()
    wq = nc.declare_dram_parameter("wq", [D, NCOLS], MMDT, isOutput=False).ap()
    wk = nc.declare_dram_parameter("wk", [D, NCOLS], MMDT, isOutput=False).ap()
    wv = nc.declare_dram_parameter("wv", [D, NCOLS], MMDT, isOutput=False).ap()
    wo = nc.declare_dram_parameter("wo", [DK, HG, D], MMDT, isOutput=False).ap()
    bq = nc.declare_dram_parameter("bq", [1, NCOLS], MMDT, isOutput=False).ap()
    bk = nc.declare_dram_parameter("bk", [1, NCOLS], MMDT, isOutput=False).ap()
    bv = nc.declare_dram_parameter("bv", [1, NCOLS], MMDT, isOutput=False).ap()
    ones_row_d = nc.declare_dram_parameter("ones_row", [1, 512], MMDT, isOutput=False).ap()
    onesc_d = nc.declare_dram_parameter("onesc", [128, DK], MMDT, isOutput=False).ap()
    cosT = nc.declare_dram_parameter("cosT", [128, S], F32, isOutput=False).ap()
    sinT = nc.declare_dram_parameter("sinT", [128, S], F32, isOutput=False).ap()
    out = nc.declare_dram_parameter("out", [S, D], F32, isOutput=True).ap()

    SB = 512            # sq block width
    NSB = S // SB       # 4
    NST = S // 128      # 16 key tiles / V tiles
    NDC = D // 128      # 8 contraction chunks
    GW = 2              # key tiles per score-psum group

    with tile.TileContext(nc) as tc, ExitStack() as ctx:
        nc.gpsimd.load_library(library_config.proxy)
        const = ctx.enter_context(tc.tile_pool(name="const", bufs=1))
        sbig = ctx.enter_context(tc.tile_pool(name="sbig", bufs=1))
        xts = ctx.enter_context(tc.tile_pool(name="xts", bufs=4))
        rtmp = ctx.enter_context(tc.tile_pool(name="rtmp", bufs=2))
        epool = ctx.enter_context(tc.tile_pool(name="epool", bufs=3))
        npool = ctx.enter_context(tc.tile_pool(name="npool", bufs=3))
        opool = ctx.enter_context(tc.tile_pool(name="opool", bufs=2))

        # --- constants / weights resident in SBUF (per-dc tiles: finer deps,
        # so the first projection matmuls start after ~128KB of DMA)
        wq_sb = [const.tile([128, NCOLS], MMDT, tag=f"wq{dc}", name=f"wq{dc}")
                 for dc in range(NDC)]
        wk_sb = [const.tile([128, NCOLS], MMDT, tag=f"wk{dc}", name=f"wk{dc}")
                 for dc in range(NDC)]
        wv_sb = [const.tile([128, NCOLS], MMDT, tag=f"wv{dc}", name=f"wv{dc}")
                 for dc in range(NDC)]
        for dc in range(NDC):
            nc.sync.dma_start(wq_sb[dc][:], wq[128 * dc:128 * dc + 128, :])
            nc.sync.dma_start(wk_sb[dc][:], wk[128 * dc:128 * dc + 128, :])
        cos_sb = const.tile([128, S], F32, tag="cos")
        sin_sb = const.tile([128, S], F32, tag="sin")
        nc.sync.dma_start(cos_sb[:], cosT)
        nc.sync.dma_start(sin_sb[:], sinT)
        for dc in range(NDC):
            nc.sync.dma_start(wv_sb[dc][:], wv[128 * dc:128 * dc + 128, :])
        # wo padded to K=128 with zero rows 64-127: fp32r matmuls with K=64
        # stream at ~2 cycles/row (HW-measured), K=128 at 1 -- zero-padding
        # the contraction nearly halves scores/Wo PE time. DMA'd after the
        # critical-path inputs (only needed in the Wo phase).
        wo_sb = const.tile([128, HG, D], MMDT, tag="wo")
        nc.sync.dma_start(wo_sb[0:DK, :, :], wo)
        for a in range(2):
            nc.vector.tensor_scalar_mul(
                wo_sb[DK:128, 2 * a:2 * a + 2, :],
                sin_sb[DK:128, :].rearrange("p (a b) -> p a b", a=2), 0.0)
        if with_bias:
            bq_sb = const.tile([1, NCOLS], MMDT, tag="bq")
            bk_sb = const.tile([1, NCOLS], MMDT, tag="bk")
            bv_sb = const.tile([1, NCOLS], MMDT, tag="bv")
            nc.sync.dma_start(bq_sb[:], bq)
            nc.sync.dma_start(bk_sb[:], bk)
            nc.sync.dma_start(bv_sb[:], bv)
        ones_row = const.tile([1, SB], MMDT, tag="ones_row")
        nc.sync.dma_start(ones_row[:], ones_row_d)
        onesc_sb = const.tile([128, DK], MMDT, tag="onesc")
        nc.sync.dma_start(onesc_sb[:], onesc_d)

        # Q^T / K^T per (chunk, sq-block): chunk c holds heads {2c, 2c+1}
        qt = [[sbig.tile([128, SB], MMDT, tag=f"qt{c}_{sb}", name=f"qt{c}_{sb}")
               for sb in range(NSB)] for c in range(2)]
        # per-head K^T, zero-padded to 128 partitions (head data on its chunk
        # rows, the complementary 64 rows zeroed)
        kth = [[sbig.tile([128, SB], MMDT, tag=f"kh{h}_{sb}", name=f"kh{h}_{sb}")
                for sb in range(NSB)] for h in range(HG)]
        for h in range(HG):
            zrows = slice(DK, 128) if h % 2 == 0 else slice(0, DK)
            for sb in range(NSB):
                nc.vector.tensor_scalar_mul(kth[h][sb][zrows, :],
                                            cos_sb[zrows, 0:SB], 0.0)
        # V augmented with a ones column per head, per key tile. Head stride
        # padded 65 -> 68 columns so each head's lhsT starts 16B-aligned.
        AUGW = DK + 4
        vaug = [sbig.tile([128, HG * AUGW], MMDT, tag=f"va{st}", name=f"va{st}")
                for st in range(NST)]
        # unnormalized O^T per (head, sq-block), zero-padded to 128 rows
        ot = [[sbig.tile([128, SB], MMDT, tag=f"ot{h}_{j}", name=f"ot{h}_{j}")
               for j in range(NSB)] for h in range(HG)]
        for h in range(HG):
            for j in range(NSB):
                nc.vector.tensor_scalar_mul(ot[h][j][DK:128, :],
                                             cos_sb[DK:128, 0:SB], 0.0)

        # ------------------------------------------------------- projections
        with tc.tile_pool(name="pj_ps", bufs=4, space="PSUM") as pj_ps, \
             tc.tile_pool(name="pv_ps", bufs=4, space="PSUM") as pvp_ps:
            for sb in range(NSB):
                ss = slice(SB * sb, SB * sb + SB)
                xt_t = []
                for dc in range(NDC):
                    t = xts.tile([128, SB], MMDT, tag="xt")
                    nc.sync.dma_start(t[:], xT[128 * dc:128 * dc + 128, ss])
                    xt_t.append(t)
                for c in range(2):
                    ncol = slice(128 * c, 128 * c + 128)
                    for (w_sb, bname) in ((wq_sb, "bq"), (wk_sb, "bk")):
                        ps = pj_ps.tile([128, SB], F32, tag="qk")
                        for dc in range(NDC):
                            nc.tensor.matmul(ps[:], w_sb[dc][:, ncol], xt_t[dc][:],
                                             start=(dc == 0),
                                             stop=(dc == NDC - 1 and not with_bias))
                        if with_bias:
                            b_sb = bq_sb if bname == "bq" else bk_sb
                            nc.tensor.matmul(ps[:], b_sb[0:1, ncol], ones_row[0:1, :],
                                             start=False, stop=True)
                        # rope: dst = ps*cos + shuffle(ps)*sin
                        t_cos = rtmp.tile([128, SB], F32, tag="rc")
                        nc.vector.tensor_mul(t_cos[:], ps[:], cos_sb[:, ss])
                        t_shuf = rtmp.tile([128, SB], F32, tag="rs")
                        nc.vector.stream_shuffle(t_shuf[:], ps[:], SHUF_MASK)
                        t_sin = rtmp.tile([128, SB], F32, tag="rm")
                        nc.gpsimd.tensor_mul(t_sin[:], t_shuf[:], sin_sb[:, ss])
                        if bname == "bq":
                            nc.vector.tensor_add(qt[c][sb][:], t_cos[:], t_sin[:])
                        else:
                            nc.vector.tensor_add(kth[2 * c][sb][0:DK, :],
                                                 t_cos[0:DK, :], t_sin[0:DK, :])
                            nc.vector.tensor_add(kth[2 * c + 1][sb][DK:128, :],
                                                 t_cos[DK:128, :], t_sin[DK:128, :])
                for st4 in range(SB // 128):
                    st = (SB // 128) * sb + st4
                    ps = pvp_ps.tile([128, NCOLS], F32, tag="v")
                    for dc in range(NDC):
                        nc.tensor.matmul(ps[:], xt_t[dc][:, 128 * st4:128 * st4 + 128],
                                         wv_sb[dc][:],
                                         start=(dc == 0),
                                         stop=(dc == NDC - 1 and not with_bias))
                    if with_bias:
                        nc.tensor.matmul(ps[:], ones_row[0:1, 0:128], bv_sb[0:1, :],
                                         start=False, stop=True)
                    # scatter heads into the augmented layout; even heads get
                    # [V | ones], odd heads [ones | V] (so PV psum offset 63
                    # puts their output on partitions 64-127)
                    va = vaug[st][:].rearrange("p (h e) -> p h e", h=HG)
                    nc.vector.tensor_copy(va[:, :, 0:DK],
                                          ps[:].rearrange("p (h k) -> p h k", h=HG))
                    nc.vector.tensor_copy(va[:, :, DK], onesc_sb[:, 0:HG])

        # -------------------------------------------------------- attention
        # S^T layout: psum group = GW key tiles x one sq block; exp on ACT;
        # PV accumulates (V | ones) so row 64 is the softmax denominator.
        with tc.tile_pool(name="sc_ps", bufs=2, space="PSUM") as sc_ps, \
             tc.tile_pool(name="o_ps", bufs=2, space="PSUM") as o_ps, \
             tc.tile_pool(name="bc_ps", bufs=2, space="PSUM") as bc_ps:
            for j in range(NSB):
                sq = slice(SB * j, SB * j + SB)
                for h in range(HG):
                    c, half = divmod(h, 2)
                    rows = slice(DK * half, DK * half + DK)
                    pv = o_ps.tile([128, SB], F32, tag="pv")
                    ngrp = (4 * j + 4) // GW
                    for g in range(ngrp):
                        sc = sc_ps.tile([128, GW * SB], F32, tag="sc")
                        for t in range(GW):
                            i = GW * g + t
                            nc.tensor.matmul(
                                sc[:, SB * t:SB * t + SB],
                                kth[h][i // 4][:, 128 * (i % 4):128 * (i % 4) + 128],
                                qt[c][j][:],
                                start=True, stop=True)
                        e = epool.tile([128, GW * SB], MMDT, tag="e")
                        nc.scalar.activation(e[:], sc[:],
                                             mybir.ActivationFunctionType.Exp,
                                             scale=SCALE)
                        d0 = GW * g - 4 * j
                        if d0 + GW > 0:  # group touches the causal diagonal
                            ev = e[:].rearrange("p (t f) -> p t f", t=GW)
                            nc.gpsimd.affine_select(
                                out=ev, in_=ev,
                                compare_op=mybir.AluOpType.is_ge,
                                fill=0.0, base=-128 * d0,
                                pattern=[[-128, GW], [1, SB]],
                                channel_multiplier=-1)
                        for t in range(GW):
                            i = GW * g + t
                            lhs = vaug[i][:].rearrange("p (h e) -> p h e", h=HG)[:, h, 0:DK + 1]
                            nc.tensor.matmul(
                                pv[0:DK + 1, :], lhs, e[:, SB * t:SB * t + SB],
                                start=(g == 0 and t == 0),
                                stop=(g == ngrp - 1 and t == GW - 1))
                    # normalize: ot = pv[0:64] * broadcast(1/pv[64])
                    rec = npool.tile([128, SB], MMDT, tag="rec")
                    with nc.allow_low_precision(reason="denominator recip in tf32"):
                        nc.vector.reciprocal(rec[DK:DK + 1, :], pv[DK:DK + 1, :])
                    bcp = bc_ps.tile([DK, SB], F32, tag="bc")
                    nc.tensor.matmul(bcp[:], onesc_sb[DK:DK + 1, :],
                                     rec[DK:DK + 1, :], start=True, stop=True)
                    bc = npool.tile([DK, SB], F32, tag="bcs")
                    nc.vector.tensor_copy(bc[:], bcp[:])
                    nc.vector.tensor_mul(ot[h][j][0:DK, :], pv[0:DK, :], bc[:])

        # ------------------------------------------------- output projection
        with tc.tile_pool(name="wo_ps", bufs=4, space="PSUM") as wo_ps:
            for st in range(NST):
                rq = slice(128 * (st % 4), 128 * (st % 4) + 128)
                jb = st // 4
                for dc in range(2):
                    cols = slice(SB * dc, SB * dc + SB)
                    ps = wo_ps.tile([128, SB], F32, tag="wo")
                    for h in range(HG):
                        nc.tensor.matmul(ps[:], ot[h][jb][:, rq], wo_sb[:, h, cols],
                                         start=(h == 0), stop=(h == HG - 1))
                    o_sb = opool.tile([128, SB], F32, tag="osb")
                    if (st + dc) % 2 == 0:
                        nc.vector.tensor_copy(o_sb[:], ps[:])
                    else:
                        nc.scalar.copy(o_sb[:], ps[:])
                    nc.sync.dma_start(out[128 * st:128 * st + 128, cols], o_sb[:])

    nc.compile()
    return nc


_CACHED_NC = {}


def _get_program(with_bias=False):
    if with_bias not in _CACHED_NC:
        _CACHED_NC[with_bias] = build_program(with_bias=with_bias)
    return _CACHED_NC[with_bias]


# ---------------------------------------------------------------------------
# entry point
# ---------------------------------------------------------------------------

def kernel(x, token_position, Wq, bq, Wk, bk, Wv, bv, Wo, bo, _results=None):
    from concourse.bass_utils import run_bass_kernel_spmd

    in_maps = make_core_inputs(x, token_position, Wq, bq, Wk, bk, Wv, bv, Wo, bo)
    if _results is None:
        with_bias = any(float(np.abs(np.asarray(v)).max()) != 0.0
                        for v in (bq, bk, bv))
        nc = _get_program(with_bias=with_bias)
        res = run_bass_kernel_spmd(nc, in_maps, list(range(N_CORES)))
        _results = [res.results[i]["out"] for i in range(N_CORES)]
    bo = np.asarray(bo, dtype=np.float32)
    out = np.empty((B, S, D), dtype=np.float32)
    for b in range(B):
        acc = _results[HG * b].astype(np.float32)
        for hg in range(1, HG):
            acc = acc + _results[HG * b + hg]
        out[b] = acc + bo[None, :]
    return out



# revision 9
# speedup vs baseline: 1.7025x; 1.7025x over previous
"""Trainium2 Bass kernel: causal multi-head attention with interleaved RoPE.

Problem shapes (hardcoded): x [2, 2048, 1024], 16 heads of dk=64.
Sharding: 8 cores = 2 batches x 4 head-groups (4 heads each). Each core
computes its head-slice Q/K/V projections, RoPE, causal attention, and a
partial output through its Wo row-slice; the host sums the 4 partials per
batch and adds bo.

v2 design (single interleaved pipeline, fp16 operands):
- All matmul operands fp16 (tolerance 2e-2; fp16 keeps ~1e-3).
- Heads processed as 2 pairs per core. Scores for a pair run as two
  concurrent row-tiled matmuls (tile_position rows 0-63 / 64-127) writing
  two PSUM banks in one 512-cycle pass -- 2x score throughput vs padded-K.
- Wo contracts a stacked head-pair (128 rows), halving Wo matmuls.
- One loop over the 4 sq-blocks interleaves projection(j+1) / Wo(j) matmul
  groups into the ACT-bound attention(j) stream so the PE never drains.
- Causal: only the needed q-column range of each diagonal score tile is
  exp'd / PV'd; triangular masking only on the 128-col diagonal band.
- Softmax denominator rides as a 65th "ones" column of V (PV psum row 64);
  reciprocal via DVE reciprocal_approx_fast, broadcast via gpsimd
  partition_broadcast (no PE/PSUM involvement).

RoPE trick: attention scores are invariant to any permutation of the dk
axis applied to both Q and K, so the Wq/Wk columns are permuted on the host
into a "quadrant half-split" layout where each rotation pair partner sits
exactly 16 partitions away inside the same 32-partition quadrant. The DVE
stream_shuffle (a per-quadrant 32-way permute) then produces the swapped
operand, and RoPE becomes: rot = q * cosT + shuffle(q) * sinT with
host-precomputed tables (sinT carries the sign).
"""

from contextlib import ExitStack

import numpy as np

import concourse.bass as bass
import concourse.mybir as mybir
import concourse.tile as tile

B, S, D, H = 2, 2048, 1024, 16
DK = D // H  # 64
HG = 4  # heads per core
NCOLS = HG * DK  # 256 columns of the projection per core
THETA = 10000.0
SCALE = 1.0 / float(np.sqrt(DK))
N_CORES = 8

F32 = mybir.dt.float32
F16 = mybir.dt.float16

SB = 512            # sq block width
NSB = S // SB       # 4
NST = S // 128      # 16 key tiles
NDC = D // 128      # 8 contraction chunks
AUGW = 72           # per-head stride in vaug (64 V cols + ones col + pad)


def f16(a):
    return np.ascontiguousarray(a, dtype=np.float16)


# ---------------------------------------------------------------------------
# host-side prep
# ---------------------------------------------------------------------------

def _rope_perm():
    """Within-head column permutation pi: new row r -> original dk index."""
    perm = np.empty(DK, dtype=np.int64)
    for r in range(DK):
        q, m = divmod(r, 32)
        if m < 16:
            perm[r] = 2 * (16 * q + m)
        else:
            perm[r] = 2 * (16 * q + m - 16) + 1
    return perm


_PERM = _rope_perm()
SHUF_MASK = list(range(16, 32)) + list(range(16))  # swap 16-halves per quadrant


def _shuf128(v):
    """Apply the quadrant shuffle to a length-128 vector (host mirror)."""
    out = np.empty_like(v)
    for q in range(4):
        out[32 * q:32 * q + 32] = v[32 * q:32 * q + 32][SHUF_MASK]
    return out


def _rope_tables(pos):
    """cosT/sinT [128, S] for the permuted layout. pos: [S] int."""
    inv_freq = (np.float32(THETA) ** (-(np.arange(0, DK, 2, dtype=np.float32) / np.float32(DK))))  # [32]
    ang = pos.astype(np.float32)[:, None] * inv_freq[None, :]  # [S, 32]
    cos = np.cos(ang)  # [S, 32]
    sin = np.sin(ang)
    cosT = np.empty((128, S), dtype=np.float32)
    sinT = np.empty((128, S), dtype=np.float32)
    for p in range(128):
        r = p % DK
        q, m = divmod(r, 32)
        if m < 16:
            i = 16 * q + m
            sgn = -1.0
        else:
            i = 16 * q + m - 16
            sgn = 1.0
        cosT[p] = cos[:, i]
        sinT[p] = np.float32(sgn) * sin[:, i]
    return cosT, sinT


def make_core_inputs(x, token_position, Wq, bq, Wk, bk, Wv, bv, Wo, bo):
    """Build the 8 per-core input maps. Returns (in_maps, with_bias)."""
    x = np.asarray(x, dtype=np.float32)
    token_position = np.asarray(token_position)
    Wq, Wk, Wv, Wo = (np.asarray(w, dtype=np.float32) for w in (Wq, Wk, Wv, Wo))
    bq, bk, bv = (np.asarray(b_, dtype=np.float32) for b_ in (bq, bk, bv))
    with_bias = any(float(np.abs(v).max()) != 0.0 for v in (bq, bk, bv))

    in_maps = []
    tables = {}
    for c in range(N_CORES):
        b, hg = divmod(c, HG)
        heads = range(HG * hg, HG * hg + HG)
        # permuted q/k column indices for this core's heads
        cols_qk = np.concatenate([DK * h + _PERM for h in heads])
        cols_v = np.arange(NCOLS * hg, NCOLS * hg + NCOLS)
        if b not in tables:
            tables[b] = _rope_tables(np.asarray(token_position[b]))
        cosT, sinT = tables[b]
        # Wo rows stacked per head pair: wo[r, p, :] = Wo row of
        # (local head 2p + r//64, dk r%64)
        wo_rows = Wo[cols_v, :].reshape(2, 2 * DK, D)  # [pair, 128, D]
        m = {
            "xT": f16(x[b].T),                               # [1024, 2048]
            "wq": f16(Wq[:, cols_qk]),                       # [1024, 256]
            "wk": f16(Wk[:, cols_qk]),
            "wv": f16(Wv[:, cols_v]),
            "wo": f16(wo_rows.transpose(1, 0, 2)),           # [128, 2, 1024]
            "cosT": f16(cosT),
            "sinT": f16(sinT),
        }
        if with_bias:
            # rope is linear: rope(q + b) = rope(q) + rope(b); rope(b) is a
            # position-dependent table added after the rope combine.
            for nm, bb in (("bqr", bq[cols_qk]), ("bkr", bk[cols_qk])):
                chunks = []
                for half in range(2):
                    blk = bb[128 * half:128 * half + 128]  # [128]
                    tbl = blk[:, None] * cosT + _shuf128(blk)[:, None] * sinT
                    chunks.append(f16(tbl))
                m[nm] = np.stack(chunks, axis=0)  # [2, 128, S]
            m["bvb"] = f16(np.broadcast_to(bv[cols_v][None, :], (128, NCOLS)).copy())
        in_maps.append(m)
    return in_maps, with_bias


# ---------------------------------------------------------------------------
# device program
# ---------------------------------------------------------------------------

def build_program(with_bias=False, dbg=False):
    from concourse import bacc, library_config
    nc = bacc.Bacc("TRN2", debug=False)

    xT = nc.declare_dram_parameter("xT", [D, S], F16, isOutput=False).ap()
    wq = nc.declare_dram_parameter("wq", [D, NCOLS], F16, isOutput=False).ap()
    wk = nc.declare_dram_parameter("wk", [D, NCOLS], F16, isOutput=False).ap()
    wv = nc.declare_dram_parameter("wv", [D, NCOLS], F16, isOutput=False).ap()
    wo = nc.declare_dram_parameter("wo", [128, 2, D], F16, isOutput=False).ap()
    cosT = nc.declare_dram_parameter("cosT", [128, S], F16, isOutput=False).ap()
    sinT = nc.declare_dram_parameter("sinT", [128, S], F16, isOutput=False).ap()
    if with_bias:
        bqr = nc.declare_dram_parameter("bqr", [2, 128, S], F16, isOutput=False).ap()
        bkr = nc.declare_dram_parameter("bkr", [2, 128, S], F16, isOutput=False).ap()
        bvb = nc.declare_dram_parameter("bvb", [128, NCOLS], F16, isOutput=False).ap()
    out = nc.declare_dram_parameter("out", [S, D], F16, isOutput=True).ap()
    if dbg:
        dbg_t = {}
        for nm, shp, dt in (("dbg_qt", [128, SB], F16), ("dbg_kt", [128, SB], F16),
                            ("dbg_va", [128, HG * AUGW], F16),
                            ("dbg_e", [128, 2 * SB], F16),
                            ("dbg_den", [128, SB], F32),
                            ("dbg_rec", [128, SB], F32),
                            ("dbg_rbc", [128, SB], F32),
                            ("dbg_ot", [128, SB], F16)):
            dbg_t[nm] = nc.declare_dram_parameter(nm, shp, dt, isOutput=True).ap()

    with tile.TileContext(nc) as tc, ExitStack() as ctx:
        nc.gpsimd.load_library(library_config.proxy)
        const = ctx.enter_context(tc.tile_pool(name="const", bufs=1))
        sbig = ctx.enter_context(tc.tile_pool(name="sbig", bufs=1))
        xts = ctx.enter_context(tc.tile_pool(name="xts", bufs=16))
        rtmp = ctx.enter_context(tc.tile_pool(name="rtmp", bufs=2))
        epool = ctx.enter_context(tc.tile_pool(name="epool", bufs=3))
        npool = ctx.enter_context(tc.tile_pool(name="npool", bufs=2))
        opool = ctx.enter_context(tc.tile_pool(name="opool", bufs=3))
        ps_big = ctx.enter_context(tc.tile_pool(name="ps_big", bufs=2, space="PSUM"))
        ps_pv = ctx.enter_context(tc.tile_pool(name="ps_pv", bufs=2, space="PSUM"))
        ps_pj = ctx.enter_context(tc.tile_pool(name="ps_pj", bufs=2, space="PSUM"))

        # --- weights / tables resident in SBUF (per-dc tiles: finer deps,
        # so the first projection matmuls start after ~64KB of DMA)
        wq_sb = [const.tile([128, NCOLS], F16, tag=f"wq{dc}", name=f"wq{dc}")
                 for dc in range(NDC)]
        wk_sb = [const.tile([128, NCOLS], F16, tag=f"wk{dc}", name=f"wk{dc}")
                 for dc in range(NDC)]
        wv_sb = [const.tile([128, NCOLS], F16, tag=f"wv{dc}", name=f"wv{dc}")
                 for dc in range(NDC)]
        for dc in range(NDC):
            nc.sync.dma_start(wq_sb[dc][:], wq[128 * dc:128 * dc + 128, :])
            nc.sync.dma_start(wk_sb[dc][:], wk[128 * dc:128 * dc + 128, :])
        cos_sb = const.tile([128, S], F16, tag="cos")
        sin_sb = const.tile([128, S], F16, tag="sin")
        nc.sync.dma_start(cos_sb[:], cosT)
        nc.sync.dma_start(sin_sb[:], sinT)
        for dc in range(NDC):
            nc.sync.dma_start(wv_sb[dc][:], wv[128 * dc:128 * dc + 128, :])
        wo_sb = const.tile([128, 2, D], F16, tag="wo")
        nc.sync.dma_start(wo_sb[:], wo)
        if with_bias:
            bqr_t = [const.tile([128, S], F16, tag=f"bqr{c}", name=f"bqr{c}")
                     for c in range(2)]
            bkr_t = [const.tile([128, S], F16, tag=f"bkr{c}", name=f"bkr{c}")
                     for c in range(2)]
            for c in range(2):
                nc.sync.dma_start(bqr_t[c][:], bqr[c])
                nc.sync.dma_start(bkr_t[c][:], bkr[c])
            bvb_sb = const.tile([128, NCOLS], F16, tag="bvb")
            nc.sync.dma_start(bvb_sb[:], bvb)

        # rope'd K^T per (chunk, block); resident for the whole sequence
        kth = [[sbig.tile([128, SB], F16, tag=f"kh{c}_{sb}", name=f"kh{c}_{sb}")
                for sb in range(NSB)] for c in range(2)]
        # V augmented with a ones column per head, per key tile
        vaug = [sbig.tile([128, HG * AUGW], F16, tag=f"va{st}", name=f"va{st}")
                for st in range(NST)]
        for st in range(NST):
            va = vaug[st][:].rearrange("p (h e) -> p h e", h=HG)
            nc.gpsimd.memset(va[:, :, DK], 1.0)

        # ------------------------------------------------------------------
        # helpers
        # ------------------------------------------------------------------

        def emit_proj(j):
            """Allocate block-j projection outputs and return (qt_pair, ops):
            ops is a list of closures, each issuing one PE matmul group plus
            its vector consumers."""
            ss = slice(SB * j, SB * j + SB)
            xt_t = []
            for dc in range(NDC):
                t = xts.tile([128, SB], F16, tag="xt", name=f"xt{j}_{dc}")
                nc.sync.dma_start(t[:], xT[128 * dc:128 * dc + 128, ss])
                xt_t.append(t)
            # rope'd Q^T pair tiles for this block (2-buf rotation)
            qt_pair = [sbig.tile([128, SB], F16, tag=f"qt{c}", name=f"qt{j}_{c}",
                                 bufs=2) for c in range(2)]

            ops = []

            def qk_group(c, kind):
                def run():
                    w_sb = wq_sb if kind == "q" else wk_sb
                    ncol = slice(128 * c, 128 * c + 128)
                    ps = ps_pj.tile([128, SB], F32, tag="pj", name="psqk")
                    for dc in range(NDC):
                        nc.tensor.matmul(ps[:], w_sb[dc][:, ncol], xt_t[dc][:],
                                         start=(dc == 0), stop=(dc == NDC - 1))
                    # rope: dst = ps*cos + shuffle(ps)*sin (+ bias table)
                    t_cos = rtmp.tile([128, SB], F16, tag="rc", name="tcos")
                    nc.vector.tensor_mul(t_cos[:], ps[:], cos_sb[:, ss])
                    t_shuf = rtmp.tile([128, SB], F32, tag="rs", name="tshuf")
                    nc.vector.stream_shuffle(t_shuf[:], ps[:], SHUF_MASK)
                    t_sin = rtmp.tile([128, SB], F16, tag="rm", name="tsin")
                    nc.gpsimd.tensor_mul(t_sin[:], t_shuf[:], sin_sb[:, ss])
                    dst = qt_pair[c] if kind == "q" else kth[c][j]
                    nc.vector.tensor_add(dst[:], t_cos[:], t_sin[:])
                    if with_bias:
                        bt = bqr_t[c] if kind == "q" else bkr_t[c]
                        nc.vector.tensor_add(dst[:], dst[:], bt[:, ss])
                return run

            def v_group(st4):
                def run():
                    st = 4 * j + st4
                    ps = ps_pj.tile([128, SB], F32, tag="pj", name="psv")
                    for dc in range(NDC):
                        nc.tensor.matmul(ps[:, 0:NCOLS],
                                         xt_t[dc][:, 128 * st4:128 * st4 + 128],
                                         wv_sb[dc][:],
                                         start=(dc == 0), stop=(dc == NDC - 1))
                    va = vaug[st][:].rearrange("p (h e) -> p h e", h=HG)
                    psv = ps[:, 0:NCOLS].rearrange("p (h k) -> p h k", h=HG)
                    if with_bias:
                        bvv = bvb_sb[:].rearrange("p (h k) -> p h k", h=HG)
                        nc.vector.tensor_add(va[:, :, 0:DK], psv, bvv)
                    else:
                        nc.vector.tensor_copy(va[:, :, 0:DK], psv)
                return run

            for c in range(2):
                ops.append(qk_group(c, "q"))
                ops.append(qk_group(c, "k"))
            for st4 in range(4):
                ops.append(v_group(st4))
            return qt_pair, ops

        def emit_wo(j, ot_pair):
            """Wo matmul groups for block j (reads the ot pair tiles)."""
            def grp(rq):
                def run():
                    ps = ps_big.tile([128, 2 * SB], F32, tag="sc", name="pswo")
                    for half in range(2):
                        cols = slice(SB * half, SB * half + SB)
                        nc.tensor.matmul(ps[:, cols],
                                         ot_pair[0][:, 128 * rq:128 * rq + 128],
                                         wo_sb[:, 0, cols], start=True, stop=False)
                        nc.tensor.matmul(ps[:, cols],
                                         ot_pair[1][:, 128 * rq:128 * rq + 128],
                                         wo_sb[:, 1, cols], start=False, stop=True)
                    o_sb = opool.tile([128, 2 * SB], F16, tag="osb", name="osb")
                    if rq % 2 == 0:
                        nc.vector.tensor_copy(o_sb[:], ps[:])
                    else:
                        nc.scalar.copy(o_sb[:], ps[:])
                    r0 = SB * j + 128 * rq
                    nc.sync.dma_start(out[r0:r0 + 128, :], o_sb[:])
                return run

            return [grp(rq) for rq in range(4)]

        # ------------------------------------------------------------------
        # main pipeline over sq blocks
        # ------------------------------------------------------------------

        # filler: list of (due_block, closure). A filler op must have run
        # before the attention of block `due_block` is emitted.
        filler = []

        qt_pair, ops = emit_proj(0)
        for op in ops:  # block 0 projections run up front
            op()

        for j in range(NSB):
            # anything due by this block runs now (normally already drained)
            while filler and filler[0][0] <= j:
                filler.pop(0)[1]()
            if j + 1 < NSB:
                nqt, ops = emit_proj(j + 1)
                filler.extend((j + 1, op) for op in ops)
            else:
                nqt = None

            ot_pair = [sbig.tile([128, SB], F16, tag=f"ot{c}", name=f"ot{j}_{c}",
                                 bufs=2) for c in range(2)]

            for c in range(2):
                pvA = ps_pv.tile([128, SB], F32, tag="pv", name="pvA")
                pvB = ps_pv.tile([128, SB], F32, tag="pv", name="pvB")
                n_tiles = 4 * j + 4
                for i in range(n_tiles):
                    kb, t = divmod(i, 4)
                    m = i - 4 * j
                    kt = kth[c][kb]
                    sc = ps_big.tile([128, 2 * SB], F32, tag="sc", name="sc")
                    nc.tensor.matmul(sc[:, 0:SB],
                                     kt[0:64, 128 * t:128 * t + 128],
                                     qt_pair[c][0:64, :],
                                     start=True, stop=True)
                    nc.tensor.matmul(sc[:, SB:2 * SB],
                                     kt[64:128, 128 * t:128 * t + 128],
                                     qt_pair[c][64:128, :],
                                     start=True, stop=True)
                    e = epool.tile([128, 2 * SB], F16, tag="e", name="e")
                    nc.scalar.activation(e[:], sc[:],
                                         mybir.ActivationFunctionType.Exp,
                                         scale=SCALE)
                    if m >= 0:  # diagonal tile: zero q cols above the diagonal
                        ev = e[:].rearrange("p (g f) -> p g f", g=2)
                        band = ev[:, :, 0:128 * (m + 1)]
                        nc.gpsimd.affine_select(
                            out=band, in_=band,
                            compare_op=mybir.AluOpType.is_ge,
                            fill=0.0, base=-128 * m,
                            pattern=[[0, 2], [1, 128 * (m + 1)]],
                            channel_multiplier=-1)
                    if dbg and j == 0 and c == 0 and i == 0:
                        nc.sync.dma_start(dbg_t["dbg_e"][:], e[:])
                    va = vaug[i][:].rearrange("p (h e) -> p h e", h=HG)
                    nc.tensor.matmul(pvA[0:DK + 1, :],
                                     va[:, 2 * c, 0:DK + 1], e[:, 0:SB],
                                     start=(i == 0), stop=(i == n_tiles - 1))
                    nc.tensor.matmul(pvB[0:DK + 1, :],
                                     va[:, 2 * c + 1, 0:DK + 1],
                                     e[:, SB:2 * SB],
                                     start=(i == 0), stop=(i == n_tiles - 1))
                    # drain one filler PE group per key tile
                    if filler:
                        filler.pop(0)[1]()
                # normalize: ot rows = pv[0:64] * broadcast(1/pv[64])
                for half, pv in ((0, pvA), (1, pvB)):
                    # custom-DVE ops misread PSUM at base_partition 64 on HW:
                    # stage the denominator row to SBUF partition 0 first
                    dsb = npool.tile([1, SB], F32, tag="dsb", name="dsb")
                    nc.vector.tensor_copy(dsb[0:1, :], pv[DK:DK + 1, :])
                    rec = npool.tile([1, SB], F32, tag="rec", name="rec")
                    nc.vector.reciprocal_approx_fast(rec[0:1, :], dsb[0:1, :])
                    rbc = npool.tile([DK, SB], F32, tag="rbc", name="rbc")
                    nc.gpsimd.partition_broadcast(rbc[0:DK, :], rec[0:1, :])
                    rows = slice(DK * half, DK * half + DK)
                    if dbg and j == 0 and c == 0 and half == 0:
                        den = npool.tile([1, SB], F32, tag="den", name="den")
                        nc.vector.tensor_copy(den[0:1, :], pv[DK:DK + 1, :])
                        nc.sync.dma_start(dbg_t["dbg_den"][0:1, :], den[0:1, :])
                        nc.sync.dma_start(dbg_t["dbg_rec"][0:1, :], rec[0:1, :])
                        nc.sync.dma_start(dbg_t["dbg_rbc"][0:DK, :], rbc[0:DK, :])
                    nc.vector.tensor_mul(ot_pair[c][rows, :], pv[0:DK, :],
                                         rbc[0:DK, :])
                if dbg and j == 0 and c == 0:
                    nc.sync.dma_start(dbg_t["dbg_qt"][:], qt_pair[0][:])
                    nc.sync.dma_start(dbg_t["dbg_kt"][:], kth[0][0][:])
                    nc.sync.dma_start(dbg_t["dbg_va"][:], vaug[0][:])
                    nc.sync.dma_start(dbg_t["dbg_ot"][:], ot_pair[0][:])

            # Wo for this block; on the last block run immediately, else queue
            wops = emit_wo(j, ot_pair)
            if j + 1 == NSB:
                for _, op in filler:
                    op()
                for op in wops:
                    op()
                filler = []
            else:
                filler.extend((j + 2, op) for op in wops)
            qt_pair = nqt

    nc.compile()
    return nc


_CACHED_NC = {}


def _get_program(with_bias=False):
    if with_bias not in _CACHED_NC:
        _CACHED_NC[with_bias] = build_program(with_bias=with_bias)
    return _CACHED_NC[with_bias]


# ---------------------------------------------------------------------------
# entry point
# ---------------------------------------------------------------------------

def kernel(x, token_position, Wq, bq, Wk, bk, Wv, bv, Wo, bo, _results=None):
    from concourse.bass_utils import run_bass_kernel_spmd

    in_maps, with_bias = make_core_inputs(
        x, token_position, Wq, bq, Wk, bk, Wv, bv, Wo, bo)
    if _results is None:
        nc = _get_program(with_bias=with_bias)
        res = run_bass_kernel_spmd(nc, in_maps, list(range(N_CORES)))
        _results = [res.results[i]["out"] for i in range(N_CORES)]
    bo = np.asarray(bo, dtype=np.float32)
    out = np.empty((B, S, D), dtype=np.float32)
    for b in range(B):
        acc = _results[HG * b].astype(np.float32)
        for hg in range(1, HG):
            acc = acc + _results[HG * b + hg].astype(np.float32)
        out[b] = acc + bo[None, :]
    return out


# revision 16
# speedup vs baseline: 1.8300x; 1.0749x over previous
"""Trainium2 Bass kernel: causal multi-head attention with interleaved RoPE.

Problem shapes (hardcoded): x [2, 2048, 1024], 16 heads of dk=64.
Sharding: 8 cores = 2 batches x 4 head-groups (4 heads each). Each core
computes its head-slice Q/K/V projections, RoPE, causal attention, and a
partial output through its Wo row-slice; the host sums the 4 partials per
batch and adds bo.

v2 design (single interleaved pipeline, fp16 operands):
- All matmul operands fp16 (tolerance 2e-2; fp16 keeps ~1e-3).
- Heads processed as 2 pairs per core. Scores for a pair run as two
  concurrent row-tiled matmuls (tile_position rows 0-63 / 64-127) writing
  two PSUM banks in one 512-cycle pass -- 2x score throughput vs padded-K.
- Wo contracts a stacked head-pair (128 rows), halving Wo matmuls.
- One loop over the 4 sq-blocks interleaves projection(j+1) / Wo(j) matmul
  groups into the ACT-bound attention(j) stream so the PE never drains.
- Causal: only the needed q-column range of each diagonal score tile is
  exp'd / PV'd; triangular masking only on the 128-col diagonal band.
- Softmax denominator rides as a 65th "ones" column of V (PV psum row 64);
  reciprocal via DVE reciprocal_approx_fast, broadcast via gpsimd
  partition_broadcast (no PE/PSUM involvement).

RoPE trick: attention scores are invariant to any permutation of the dk
axis applied to both Q and K, so the Wq/Wk columns are permuted on the host
into a "quadrant half-split" layout where each rotation pair partner sits
exactly 16 partitions away inside the same 32-partition quadrant. The DVE
stream_shuffle (a per-quadrant 32-way permute) then produces the swapped
operand, and RoPE becomes: rot = q * cosT + shuffle(q) * sinT with
host-precomputed tables (sinT carries the sign).
"""

from contextlib import ExitStack

import numpy as np

import concourse.bass as bass
import concourse.mybir as mybir
import concourse.tile as tile

B, S, D, H = 2, 2048, 1024, 16
DK = D // H  # 64
HG = 4  # heads per core
NCOLS = HG * DK  # 256 columns of the projection per core
THETA = 10000.0
SCALE = 1.0 / float(np.sqrt(DK))
N_CORES = 8

F32 = mybir.dt.float32
F16 = mybir.dt.float16

SB = 512            # sq block width
NSB = S // SB       # 4
NST = S // 128      # 16 key tiles
NDC = D // 128      # 8 contraction chunks
AUGW = 72           # per-head stride in vaug (64 V cols + ones col + pad)
SLICED = True       # restrict diagonal score tiles to the causal q-cols


def f16(a):
    return np.ascontiguousarray(a, dtype=np.float16)


# ---------------------------------------------------------------------------
# host-side prep
# ---------------------------------------------------------------------------

def _rope_perm():
    """Within-head column permutation pi: new row r -> original dk index."""
    perm = np.empty(DK, dtype=np.int64)
    for r in range(DK):
        q, m = divmod(r, 32)
        if m < 16:
            perm[r] = 2 * (16 * q + m)
        else:
            perm[r] = 2 * (16 * q + m - 16) + 1
    return perm


_PERM = _rope_perm()
SHUF_MASK = list(range(16, 32)) + list(range(16))  # swap 16-halves per quadrant


def _shuf128(v):
    """Apply the quadrant shuffle to a length-128 vector (host mirror)."""
    out = np.empty_like(v)
    for q in range(4):
        out[32 * q:32 * q + 32] = v[32 * q:32 * q + 32][SHUF_MASK]
    return out


def _rope_tables(pos):
    """cosT/sinT [128, S] for the permuted layout. pos: [S] int."""
    inv_freq = (np.float32(THETA) ** (-(np.arange(0, DK, 2, dtype=np.float32) / np.float32(DK))))  # [32]
    ang = pos.astype(np.float32)[:, None] * inv_freq[None, :]  # [S, 32]
    cos = np.cos(ang)  # [S, 32]
    sin = np.sin(ang)
    cosT = np.empty((128, S), dtype=np.float32)
    sinT = np.empty((128, S), dtype=np.float32)
    for p in range(128):
        r = p % DK
        q, m = divmod(r, 32)
        if m < 16:
            i = 16 * q + m
            sgn = -1.0
        else:
            i = 16 * q + m - 16
            sgn = 1.0
        cosT[p] = cos[:, i]
        sinT[p] = np.float32(sgn) * sin[:, i]
    return cosT, sinT


def make_core_inputs(x, token_position, Wq, bq, Wk, bk, Wv, bv, Wo, bo):
    """Build the 8 per-core input maps. Returns (in_maps, with_bias)."""
    x = np.asarray(x, dtype=np.float32)
    token_position = np.asarray(token_position)
    Wq, Wk, Wv, Wo = (np.asarray(w, dtype=np.float32) for w in (Wq, Wk, Wv, Wo))
    bq, bk, bv = (np.asarray(b_, dtype=np.float32) for b_ in (bq, bk, bv))
    with_bias = any(float(np.abs(v).max()) != 0.0 for v in (bq, bk, bv))

    in_maps = []
    tables = {}
    for c in range(N_CORES):
        b, hg = divmod(c, HG)
        heads = range(HG * hg, HG * hg + HG)
        # permuted q/k column indices for this core's heads
        cols_qk = np.concatenate([DK * h + _PERM for h in heads])
        cols_v = np.arange(NCOLS * hg, NCOLS * hg + NCOLS)
        if b not in tables:
            tables[b] = _rope_tables(np.asarray(token_position[b]))
        cosT, sinT = tables[b]
        # Wo rows stacked per head pair: wo[r, p, :] = Wo row of
        # (local head 2p + r//64, dk r%64)
        wo_rows = Wo[cols_v, :].reshape(2, 2 * DK, D)  # [pair, 128, D]
        m = {
            "xT": f16(x[b].T),                               # [1024, 2048]
            "wq": f16(Wq[:, cols_qk]),                       # [1024, 256]
            "wk": f16(Wk[:, cols_qk]),
            "wv": f16(Wv[:, cols_v]),
            "wo": f16(wo_rows.transpose(1, 0, 2)),           # [128, 2, 1024]
            "cosT": f16(cosT),
            "sinT": f16(sinT),
        }
        if with_bias:
            # rope is linear: rope(q + b) = rope(q) + rope(b); rope(b) is a
            # position-dependent table added after the rope combine.
            for nm, bb in (("bqr", bq[cols_qk]), ("bkr", bk[cols_qk])):
                chunks = []
                for half in range(2):
                    blk = bb[128 * half:128 * half + 128]  # [128]
                    tbl = blk[:, None] * cosT + _shuf128(blk)[:, None] * sinT
                    chunks.append(f16(tbl))
                m[nm] = np.stack(chunks, axis=0)  # [2, 128, S]
            m["bvb"] = f16(np.broadcast_to(bv[cols_v][None, :], (128, NCOLS)).copy())
        in_maps.append(m)
    return in_maps, with_bias


# ---------------------------------------------------------------------------
# device program
# ---------------------------------------------------------------------------

def build_program(with_bias=False, dbg=False):
    from concourse import bacc, library_config
    nc = bacc.Bacc("TRN2", debug=False)

    xT = nc.declare_dram_parameter("xT", [D, S], F16, isOutput=False).ap()
    wq = nc.declare_dram_parameter("wq", [D, NCOLS], F16, isOutput=False).ap()
    wk = nc.declare_dram_parameter("wk", [D, NCOLS], F16, isOutput=False).ap()
    wv = nc.declare_dram_parameter("wv", [D, NCOLS], F16, isOutput=False).ap()
    wo = nc.declare_dram_parameter("wo", [128, 2, D], F16, isOutput=False).ap()
    cosT = nc.declare_dram_parameter("cosT", [128, S], F16, isOutput=False).ap()
    sinT = nc.declare_dram_parameter("sinT", [128, S], F16, isOutput=False).ap()
    if with_bias:
        bqr = nc.declare_dram_parameter("bqr", [2, 128, S], F16, isOutput=False).ap()
        bkr = nc.declare_dram_parameter("bkr", [2, 128, S], F16, isOutput=False).ap()
        bvb = nc.declare_dram_parameter("bvb", [128, NCOLS], F16, isOutput=False).ap()
    out = nc.declare_dram_parameter("out", [S, D], F16, isOutput=True).ap()
    if dbg:
        dbg_t = {}
        for nm, shp, dt in (("dbg_qt", [128, SB], F16), ("dbg_kt", [128, SB], F16),
                            ("dbg_va", [128, HG * AUGW], F16),
                            ("dbg_e", [128, 2 * SB], F16),
                            ("dbg_den", [128, SB], F32),
                            ("dbg_rec", [128, SB], F32),
                            ("dbg_rbc", [128, SB], F32),
                            ("dbg_ot", [128, SB], F16)):
            dbg_t[nm] = nc.declare_dram_parameter(nm, shp, dt, isOutput=True).ap()

    with tile.TileContext(nc) as tc, ExitStack() as ctx:
        nc.gpsimd.load_library(library_config.proxy)
        const = ctx.enter_context(tc.tile_pool(name="const", bufs=1))
        sbig = ctx.enter_context(tc.tile_pool(name="sbig", bufs=1))
        xts = ctx.enter_context(tc.tile_pool(name="xts", bufs=16))
        rtmp = ctx.enter_context(tc.tile_pool(name="rtmp", bufs=2))
        epool = ctx.enter_context(tc.tile_pool(name="epool", bufs=3))
        npool = ctx.enter_context(tc.tile_pool(name="npool", bufs=2))
        opool = ctx.enter_context(tc.tile_pool(name="opool", bufs=3))
        ps_big = ctx.enter_context(tc.tile_pool(name="ps_big", bufs=2, space="PSUM"))
        ps_pv = ctx.enter_context(tc.tile_pool(name="ps_pv", bufs=2, space="PSUM"))
        ps_pj = ctx.enter_context(tc.tile_pool(name="ps_pj", bufs=2, space="PSUM"))

        # --- weights / tables resident in SBUF (per-dc tiles: finer deps,
        # so the first projection matmuls start after ~64KB of DMA).
        # DMA priority order: wq/wk (first QK groups), then x block 0 (issued
        # in emit_proj(0) below), then cos/sin (rope), wv, wo.
        wq_sb = [const.tile([128, NCOLS], F16, tag=f"wq{dc}", name=f"wq{dc}")
                 for dc in range(NDC)]
        wk_sb = [const.tile([128, NCOLS], F16, tag=f"wk{dc}", name=f"wk{dc}")
                 for dc in range(NDC)]
        wv_sb = [const.tile([128, NCOLS], F16, tag=f"wv{dc}", name=f"wv{dc}")
                 for dc in range(NDC)]
        for dc in range(NDC):
            nc.sync.dma_start(wq_sb[dc][:], wq[128 * dc:128 * dc + 128, :])
            nc.sync.dma_start(wk_sb[dc][:], wk[128 * dc:128 * dc + 128, :])
        cos_sb = const.tile([128, S], F16, tag="cos")
        sin_sb = const.tile([128, S], F16, tag="sin")
        wo_sb = const.tile([128, 2, D], F16, tag="wo")
        if with_bias:
            bqr_t = [const.tile([128, S], F16, tag=f"bqr{c}", name=f"bqr{c}")
                     for c in range(2)]
            bkr_t = [const.tile([128, S], F16, tag=f"bkr{c}", name=f"bkr{c}")
                     for c in range(2)]
            bvb_sb = const.tile([128, NCOLS], F16, tag="bvb")

        # rope'd K^T per (chunk, block); resident for the whole sequence
        kth = [[sbig.tile([128, SB], F16, tag=f"kh{c}_{sb}", name=f"kh{c}_{sb}")
                for sb in range(NSB)] for c in range(2)]
        # V augmented with a ones column per head, per key tile
        vaug = [sbig.tile([128, HG * AUGW], F16, tag=f"va{st}", name=f"va{st}")
                for st in range(NST)]

        # ------------------------------------------------------------------
        # helpers
        # ------------------------------------------------------------------

        def emit_proj(j):
            """Allocate block-j projection outputs and return (qt_pair, ops):
            ops is a list of closures, each issuing one PE matmul group plus
            its vector consumers."""
            ss = slice(SB * j, SB * j + SB)
            xt_t = []
            for dc in range(NDC):
                t = xts.tile([128, SB], F16, tag="xt", name=f"xt{j}_{dc}")
                nc.sync.dma_start(t[:], xT[128 * dc:128 * dc + 128, ss])
                xt_t.append(t)
            # rope'd Q^T pair tiles for this block (2-buf rotation)
            qt_pair = [sbig.tile([128, SB], F16, tag=f"qt{c}", name=f"qt{j}_{c}",
                                 bufs=2) for c in range(2)]

            ops = []

            def qk_group(c, kind):
                def run():
                    w_sb = wq_sb if kind == "q" else wk_sb
                    ncol = slice(128 * c, 128 * c + 128)
                    ps = ps_pj.tile([128, SB], F32, tag="pj", name="psqk")
                    for dc in range(NDC):
                        nc.tensor.matmul(ps[:], w_sb[dc][:, ncol], xt_t[dc][:],
                                         start=(dc == 0), stop=(dc == NDC - 1))
                    # rope: dst = ps*cos + shuffle(ps)*sin (+ bias table)
                    t_cos = rtmp.tile([128, SB], F16, tag="rc", name="tcos")
                    nc.vector.tensor_mul(t_cos[:], ps[:], cos_sb[:, ss])
                    t_shuf = rtmp.tile([128, SB], F32, tag="rs", name="tshuf")
                    nc.vector.stream_shuffle(t_shuf[:], ps[:], SHUF_MASK)
                    t_sin = rtmp.tile([128, SB], F16, tag="rm", name="tsin")
                    nc.gpsimd.tensor_mul(t_sin[:], t_shuf[:], sin_sb[:, ss])
                    dst = qt_pair[c] if kind == "q" else kth[c][j]
                    nc.vector.tensor_add(dst[:], t_cos[:], t_sin[:])
                    if with_bias:
                        bt = bqr_t[c] if kind == "q" else bkr_t[c]
                        nc.vector.tensor_add(dst[:], dst[:], bt[:, ss])
                return run

            def v_group(st4):
                def run():
                    st = 4 * j + st4
                    ps = ps_pj.tile([128, SB], F32, tag="pj", name="psv")
                    for dc in range(NDC):
                        nc.tensor.matmul(ps[:, 0:NCOLS],
                                         xt_t[dc][:, 128 * st4:128 * st4 + 128],
                                         wv_sb[dc][:],
                                         start=(dc == 0), stop=(dc == NDC - 1))
                    va = vaug[st][:].rearrange("p (h e) -> p h e", h=HG)
                    psv = ps[:, 0:NCOLS].rearrange("p (h k) -> p h k", h=HG)
                    if with_bias:
                        bvv = bvb_sb[:].rearrange("p (h k) -> p h k", h=HG)
                        nc.vector.tensor_add(va[:, :, 0:DK], psv, bvv)
                    else:
                        nc.vector.tensor_copy(va[:, :, 0:DK], psv)
                return run

            for c in range(2):
                ops.append(qk_group(c, "q"))
                ops.append(qk_group(c, "k"))
            for st4 in range(4):
                ops.append(v_group(st4))
            return qt_pair, ops

        def emit_wo(j, ot_pair):
            """Wo matmul groups for block j (reads the ot pair tiles)."""
            def grp(rq):
                def run():
                    ps = ps_big.tile([128, 2 * SB], F32, tag="sc", name="pswo")
                    for half in range(2):
                        cols = slice(SB * half, SB * half + SB)
                        nc.tensor.matmul(ps[:, cols],
                                         ot_pair[0][:, 128 * rq:128 * rq + 128],
                                         wo_sb[:, 0, cols], start=True, stop=False)
                        nc.tensor.matmul(ps[:, cols],
                                         ot_pair[1][:, 128 * rq:128 * rq + 128],
                                         wo_sb[:, 1, cols], start=False, stop=True)
                    o_sb = opool.tile([128, 2 * SB], F16, tag="osb", name="osb")
                    nc.vector.tensor_copy(o_sb[:], ps[:])
                    r0 = SB * j + 128 * rq
                    # output DMA on the gpsimd queue so it never head-of-line
                    # blocks the input prefetches on the sync queue
                    nc.gpsimd.dma_start(out[r0:r0 + 128, :], o_sb[:])
                return run

            return [grp(rq) for rq in range(4)]

        # ------------------------------------------------------------------
        # main pipeline over sq blocks
        # ------------------------------------------------------------------

        # filler: list of (due_block, closure). A filler op must have run
        # before the attention of block `due_block` is emitted.
        filler = []

        qt_pair, ops = emit_proj(0)  # also issues block-0 x DMAs
        # lower-priority constants after the block-0 critical path
        nc.sync.dma_start(cos_sb[:], cosT)
        nc.sync.dma_start(sin_sb[:], sinT)
        for dc in range(NDC):
            nc.sync.dma_start(wv_sb[dc][:], wv[128 * dc:128 * dc + 128, :])
        nc.sync.dma_start(wo_sb[:], wo)
        if with_bias:
            for c in range(2):
                nc.sync.dma_start(bqr_t[c][:], bqr[c])
                nc.sync.dma_start(bkr_t[c][:], bkr[c])
            nc.sync.dma_start(bvb_sb[:], bvb)
        for st in range(NST):
            va = vaug[st][:].rearrange("p (h e) -> p h e", h=HG)
            nc.gpsimd.memset(va[:, :, DK], 1.0)
        for op in ops:  # block 0 projections run up front
            op()

        for j in range(NSB):
            # anything due by this block runs now (normally already drained)
            while filler and filler[0][0] <= j:
                filler.pop(0)[1]()
            if j + 1 < NSB:
                nqt, ops = emit_proj(j + 1)
                filler.extend((j + 1, op) for op in ops)
            else:
                nqt = None

            ot_pair = [sbig.tile([128, SB], F16, tag=f"ot{c}", name=f"ot{j}_{c}",
                                 bufs=2) for c in range(2)]

            for c in range(2):
                pvA = ps_pv.tile([128, SB], F32, tag="pv", name="pvA")
                pvB = ps_pv.tile([128, SB], F32, tag="pv", name="pvB")
                n_tiles = 4 * j + 4
                for i in range(n_tiles):
                    kb, t = divmod(i, 4)
                    m = i - 4 * j
                    # q columns < c0 are entirely above the causal diagonal
                    c0 = 128 * m if (SLICED and m > 0) else 0
                    kt = kth[c][kb]
                    sc = ps_big.tile([128, 2 * SB], F32, tag="sc", name="sc")
                    nc.tensor.matmul(sc[:, c0:SB],
                                     kt[0:64, 128 * t:128 * t + 128],
                                     qt_pair[c][0:64, c0:SB],
                                     start=True, stop=True)
                    nc.tensor.matmul(sc[:, SB + c0:2 * SB],
                                     kt[64:128, 128 * t:128 * t + 128],
                                     qt_pair[c][64:128, c0:SB],
                                     start=True, stop=True)
                    e = epool.tile([128, 2 * SB], F16, tag="e", name="e")
                    ev = e[:].rearrange("p (g f) -> p g f", g=2)
                    scv = sc[:].rearrange("p (g f) -> p g f", g=2)
                    nc.scalar.activation(ev[:, :, c0:SB], scv[:, :, c0:SB],
                                         mybir.ActivationFunctionType.Exp,
                                         scale=SCALE)
                    if m >= 0:  # diagonal tile: zero the triangular band
                        band = ev[:, :, c0:128 * (m + 1)]
                        nc.gpsimd.affine_select(
                            out=band, in_=band,
                            compare_op=mybir.AluOpType.is_ge,
                            fill=0.0, base=c0 - 128 * m,
                            pattern=[[0, 2], [1, 128 * (m + 1) - c0]],
                            channel_multiplier=-1)
                    if dbg and j == 0 and c == 0 and i == 0:
                        nc.sync.dma_start(dbg_t["dbg_e"][:], e[:])
                    va = vaug[i][:].rearrange("p (h e) -> p h e", h=HG)
                    nc.tensor.matmul(pvA[0:DK + 1, c0:SB],
                                     va[:, 2 * c, 0:DK + 1], e[:, c0:SB],
                                     start=(i == 0), stop=(i == n_tiles - 1))
                    nc.tensor.matmul(pvB[0:DK + 1, c0:SB],
                                     va[:, 2 * c + 1, 0:DK + 1],
                                     e[:, SB + c0:2 * SB],
                                     start=(i == 0), stop=(i == n_tiles - 1))
                    # drain one filler PE group per key tile
                    if filler:
                        filler.pop(0)[1]()
                # normalize: ot rows = pv[0:64] * broadcast(1/pv[64])
                for half, pv in ((0, pvA), (1, pvB)):
                    # custom-DVE ops misread PSUM at base_partition 64 on HW:
                    # stage the denominator row to SBUF partition 0 first
                    dsb = npool.tile([1, SB], F32, tag="dsb", name="dsb")
                    nc.vector.tensor_copy(dsb[0:1, :], pv[DK:DK + 1, :])
                    rec = npool.tile([1, SB], F32, tag="rec", name="rec")
                    nc.vector.reciprocal_approx_fast(rec[0:1, :], dsb[0:1, :])
                    rbc = npool.tile([DK, SB], F32, tag="rbc", name="rbc")
                    nc.gpsimd.partition_broadcast(rbc[0:DK, :], rec[0:1, :])
                    rows = slice(DK * half, DK * half + DK)
                    if dbg and j == 0 and c == 0 and half == 0:
                        den = npool.tile([1, SB], F32, tag="den", name="den")
                        nc.vector.tensor_copy(den[0:1, :], pv[DK:DK + 1, :])
                        nc.sync.dma_start(dbg_t["dbg_den"][0:1, :], den[0:1, :])
                        nc.sync.dma_start(dbg_t["dbg_rec"][0:1, :], rec[0:1, :])
                        nc.sync.dma_start(dbg_t["dbg_rbc"][0:DK, :], rbc[0:DK, :])
                    nc.vector.tensor_mul(ot_pair[c][rows, :], pv[0:DK, :],
                                         rbc[0:DK, :])
                if dbg and j == 0 and c == 0:
                    nc.sync.dma_start(dbg_t["dbg_qt"][:], qt_pair[0][:])
                    nc.sync.dma_start(dbg_t["dbg_kt"][:], kth[0][0][:])
                    nc.sync.dma_start(dbg_t["dbg_va"][:], vaug[0][:])
                    nc.sync.dma_start(dbg_t["dbg_ot"][:], ot_pair[0][:])

            # Wo for this block; on the last block run immediately, else queue
            wops = emit_wo(j, ot_pair)
            if j + 1 == NSB:
                for _, op in filler:
                    op()
                for op in wops:
                    op()
                filler = []
            else:
                filler.extend((j + 2, op) for op in wops)
            qt_pair = nqt

    nc.compile()
    return nc


_CACHED_NC = {}


def _get_program(with_bias=False):
    if with_bias not in _CACHED_NC:
        _CACHED_NC[with_bias] = build_program(with_bias=with_bias)
    return _CACHED_NC[with_bias]


# ---------------------------------------------------------------------------
# entry point
# ---------------------------------------------------------------------------

def kernel(x, token_position, Wq, bq, Wk, bk, Wv, bv, Wo, bo, _results=None):
    from concourse.bass_utils import run_bass_kernel_spmd

    in_maps, with_bias = make_core_inputs(
        x, token_position, Wq, bq, Wk, bk, Wv, bv, Wo, bo)
    if _results is None:
        nc = _get_program(with_bias=with_bias)
        res = run_bass_kernel_spmd(nc, in_maps, list(range(N_CORES)))
        _results = [res.results[i]["out"] for i in range(N_CORES)]
    bo = np.asarray(bo, dtype=np.float32)
    out = np.empty((B, S, D), dtype=np.float32)
    for b in range(B):
        acc = _results[HG * b].astype(np.float32)
        for hg in range(1, HG):
            acc = acc + _results[HG * b + hg].astype(np.float32)
        out[b] = acc + bo[None, :]
    return out
